# revision 51
# baseline (speedup 1.0000x reference)
"""Differential multi-head attention kernel for Trainium2 (8 NeuronCores).

v2: all-bf16 matmul pipeline, weights loaded once per core, both local
batches folded into the matmul free dimension for the projections.

Per core (data-parallel over batch: 16 / 8 = 2 local batches):
  Once:    load Wq/Wk/Wv/Wo fp32 -> cast bf16 in SBUF (lhsT layouts).
  Phase 1: x -> xT (bf16 PE transposes); QT/KT = W.T @ xT with both
           batches side-by-side in the free dim (j=1156); V -> augmented
           vaug (ones column at 64 so dual-softmax denominators fall out
           of the ctx matmul).
  Phase 2: per (b, head): scoresT[kp, q] (bf16, j splits 512/66), one
           exp per (kp, branch) on ACT -> bf16 e tiles; ctx matmuls
           accumulate [65, 578] = [ctxT; denom]; drain on Pool; bf16
           reciprocal + DMA partition-broadcast; combine on DVE (16-bit
           2x mode) with accum_out feeding GroupNorm stats.
  GN:      stats cross-partition reduce via ones-matmul; rstd =
           exp(-0.5*ln(var+eps)); per-head affine; scr (bf16 DRAM) holds
           the [768, 577] buffer whose flat reinterpret as [577, 768] is
           the torch reshape.
  Phase 3: read scr back as [577, 768] rows, PE-transpose to ctxTT,
           out = ctx @ Wo + bo (psum drained on ACT).
"""
import numpy as np

import concourse.bass as bass
import concourse.tile as tile
from concourse import mybir, bacc
from concourse import bass_utils
from concourse.masks import make_identity

f32 = mybir.dt.float32
bf16 = mybir.dt.bfloat16
AF = mybir.ActivationFunctionType
OP = mybir.AluOpType

B, S, D = 16, 577, 768
H, Dh = 12, 64
N_CORES = 8
BL = B // N_CORES          # local batches per core
SQ = 578                   # padded seq (577 + 1 pad col)
SQ2 = BL * SQ              # both batches folded in the free dim
NK = D // 128              # 6 contraction chunks of D
NC2 = 2 * D // 128         # 12 chunks of 2D (dual Q/K)
NT = (S + 127) // 128      # 5 seq tiles
LAST = S - 4 * 128         # 65 rows in the last seq tile
EPS = 1e-5
GN_N = float(Dh * S)


def bcast_ap(row_ap, nrows):
    """Partition-broadcast AP: repeat a single-partition row over nrows (DMA src)."""
    return bass.AP(tensor=row_ap.tensor, offset=row_ap.offset,
                   ap=[list(row_ap.ap[0]), [0, nrows]] + [list(x) for x in row_ap.ap[1:]])


def build_program(lam: float):
    nc = bacc.Bacc(trn_type="TRN2", target_bir_lowering=False, debug=False)

    x = nc.dram_tensor("x", [BL, S, D], f32, kind="ExternalInput").ap()
    Wq = nc.dram_tensor("Wq", [D, 2 * D], f32, kind="ExternalInput").ap()
    bq = nc.dram_tensor("bq", [2 * D], f32, kind="ExternalInput").ap()
    Wk = nc.dram_tensor("Wk", [D, 2 * D], f32, kind="ExternalInput").ap()
    bk = nc.dram_tensor("bk", [2 * D], f32, kind="ExternalInput").ap()
    Wv = nc.dram_tensor("Wv", [D, D], f32, kind="ExternalInput").ap()
    bv = nc.dram_tensor("bv", [D], f32, kind="ExternalInput").ap()
    Wo = nc.dram_tensor("Wo", [D, D], f32, kind="ExternalInput").ap()
    bo = nc.dram_tensor("bo", [D], f32, kind="ExternalInput").ap()
    gn_w = nc.dram_tensor("gn_w", [D], f32, kind="ExternalInput").ap()
    gn_b = nc.dram_tensor("gn_b", [D], f32, kind="ExternalInput").ap()
    out = nc.dram_tensor("out", [BL, S, D], f32, kind="ExternalOutput").ap()

    with tile.TileContext(nc) as tc:
        build_body(nc, tc, x, Wq, bq, Wk, bk, Wv, bv, Wo, bo, gn_w, gn_b, out, lam)
    nc.compile()
    return nc


def build_body(nc, tc, x, Wq, bq, Wk, bk, Wv, bv, Wo, bo, gn_w, gn_b, out, lam):
    # ---------- singles ----------
    sing = tc.alloc_tile_pool(name="singles", bufs=1)
    identb = sing.tile([128, 128], bf16, tag="identb", name="identb")
    make_identity(nc, identb)
    identf = sing.tile([128, 128], f32, tag="identf", name="identf")
    make_identity(nc, identf)

    bqT = sing.tile([128, NC2], f32, tag="bqT", name="bqT")
    nc.sync.dma_start(out=bqT, in_=bass.AP(tensor=bq.tensor, offset=bq.offset,
                                           ap=[[1, 128], [128, NC2]]))
    bkT = sing.tile([128, NC2], f32, tag="bkT", name="bkT")
    nc.sync.dma_start(out=bkT, in_=bass.AP(tensor=bk.tensor, offset=bk.offset,
                                           ap=[[1, 128], [128, NC2]]))
    gn_wT = sing.tile([64, H], f32, tag="gn_wT", name="gn_wT")
    nc.sync.dma_start(out=gn_wT, in_=bass.AP(tensor=gn_w.tensor, offset=gn_w.offset,
                                             ap=[[1, 64], [64, H]]))
    gn_bT = sing.tile([64, H], f32, tag="gn_bT", name="gn_bT")
    nc.sync.dma_start(out=gn_bT, in_=bass.AP(tensor=gn_b.tensor, offset=gn_b.offset,
                                             ap=[[1, 64], [64, H]]))
    eps_t = sing.tile([1, 1], f32, tag="eps_t", name="eps_t")
    nc.gpsimd.memset(eps_t, EPS)
    ones64 = sing.tile([64, 1], f32, tag="ones64", name="ones64")
    nc.gpsimd.memset(ones64, 1.0)
    onesb = sing.tile([1, 128], bf16, tag="onesb", name="onesb")
    nc.gpsimd.memset(onesb, 1.0)

    # bias rows (bf16) for the ones-column rank-1 bias matmuls
    brow = sing.tile([1, 2 * D], bf16, tag="brow", name="brow")
    bvb = brow[0:1, 0:D]
    bob = brow[0:1, D:2 * D]

    # ---------- weights: load fp32 once (contiguous rows), cast to bf16 ----------
    wpool = tc.alloc_tile_pool(name="wpool", bufs=1)
    wqkpool = tc.alloc_tile_pool(name="wqkpool", bufs=1, side="right")
    wq = wqkpool.tile([128, NK, 2 * D], bf16, tag="wq", name="wq")
    wk = wqkpool.tile([128, NK, 2 * D], bf16, tag="wk", name="wk")
    wv = wpool.tile([128, NK, D], bf16, tag="wv", name="wv")
    wo = wpool.tile([128, NK, D], bf16, tag="wo", name="wo")

    def cast_copy(eng, out, in_):
        if eng is nc.scalar:
            nc.scalar.copy(out=out, in_=in_)
        else:
            eng.tensor_copy(out=out, in_=in_)

    ca = [nc.vector, nc.scalar]

    # ---------- SBUF pools ----------
    big = tc.alloc_tile_pool(name="big", bufs=1)
    vpool = tc.alloc_tile_pool(name="vpool", bufs=1)
    epool = tc.alloc_tile_pool(name="epool", bufs=3)
    cpool = tc.alloc_tile_pool(name="cpool", bufs=2)
    rpool = tc.alloc_tile_pool(name="rpool", bufs=2)
    ctxpool = tc.alloc_tile_pool(name="ctxpool", bufs=2)
    stpool = tc.alloc_tile_pool(name="stpool", bufs=1)
    opool = tc.alloc_tile_pool(name="opool", bufs=2)
    drpool = tc.alloc_tile_pool(name="drpool", bufs=1, space="DRAM")

    # ================= Phase 1: x -> xT, weights, QK/V projections =========
    # One PSUM pool so x-transposes, QK chunks and V tiles pipeline freely.
    xT = big.tile([128, NK, SQ2], bf16, tag="xT_ctxTT", name="xT")
    QT = big.tile([128, NC2, SQ2], bf16, tag="QT", name="QT")
    KT = big.tile([128, NC2, SQ2], bf16, tag="KT", name="KT")
    vaugs = []
    for b in range(BL):
        va = vpool.tile([128, NT, H, 65], bf16, tag=f"vaug{b}", name=f"vaug{b}")
        nc.gpsimd.memset(va[:, :, :, 64:65], 1.0)
        vaugs.append(va)

    JS = [(0, 512), (512, 1024), (1024, SQ2)]
    with tc.tile_pool(name="ps_p1", bufs=1, space="PSUM") as ps1, \
         tc.tile_pool(name="wstpool", bufs=2) as wstpool, \
         tc.tile_pool(name="xapool", bufs=2) as xapool:
        # x loads first (sync queue), then weight loads (gpsimd queue) —
        # separate queues so both streams run concurrently from t=0.
        xns = []
        for b in range(BL):
            for t in range(NT):
                sz = 128 if t < NT - 1 else LAST
                xn = xapool.tile([128, D], f32, tag="xn", name=f"xn_{b}_{t}")
                if sz < 128:
                    nc.gpsimd.memset(xn[64:128, :], 0.0)
                nc.sync.dma_start(out=xn[0:sz, :], in_=x[b, t * 128:t * 128 + sz, :])
                xns.append(xn)
        brow_f = wstpool.tile([1, D], f32, tag="wst", name="brow_f")
        nc.sync.dma_start(out=brow_f,
                          in_=bass.AP(tensor=bv.tensor, offset=bv.offset,
                                      ap=[[D, 1], [1, D]]))
        nc.vector.tensor_copy(brow[0:1, 0:D], brow_f)
        brow_f2 = wstpool.tile([1, D], f32, tag="wst", name="brow_f2")
        nc.sync.dma_start(out=brow_f2,
                          in_=bass.AP(tensor=bo.tensor, offset=bo.offset,
                                      ap=[[D, 1], [1, D]]))
        nc.vector.tensor_copy(brow[0:1, D:2 * D], brow_f2)
        ei = 0
        for (W, wt) in ((Wq, wq), (Wk, wk)):
            for k in range(NK):
                for hf in range(2):
                    wst = wstpool.tile([128, D], f32, tag="wst",
                                       name=f"wst_{wt.tensor.name}_{k}_{hf}")
                    nc.gpsimd.dma_start(
                        out=wst,
                        in_=W[k * 128:(k + 1) * 128, hf * D:(hf + 1) * D])
                    cast_copy(ca[ei % 2], wt[:, k, hf * D:(hf + 1) * D], wst)
                    ei += 1

        # x transposes (PE) as soon as tiles land
        for b in range(BL):
            for t in range(NT):
                w = min(128, SQ - t * 128)
                tp = ps1.tile([128, NK, 128], f32, tag="tp", bufs=1,
                              name=f"tp_{b}_{t}")
                for k in range(NK):
                    nc.tensor.transpose(tp[:, k, :],
                                        xns[b * NT + t][:, k * 128:(k + 1) * 128],
                                        identf)
                nc.vector.tensor_copy(
                    out=xT[:, :, b * SQ + t * 128:b * SQ + t * 128 + w],
                    in_=tp[:, :, 0:w])

        # QK projections: j-chunk-outer so the first pass only needs batch-0
        # xT tiles; per-chunk PSUM is a single bank.
        for (j0, j1) in JS:
            jw = j1 - j0
            for (wt, dst, bT) in ((wq, QT, bqT), (wk, KT, bkT)):
                for c in range(NC2):
                    pp = ps1.tile([128, 512], f32, tag="pp", bufs=2,
                                  name=f"pp_{dst.tensor.name}_{c}_{j0}")
                    for k in range(NK):
                        nc.tensor.matmul(pp[:, 0:jw],
                                         wt[:, k, c * 128:(c + 1) * 128],
                                         xT[:, k, j0:j1],
                                         start=(k == 0), stop=(k == NK - 1),
                                         skip_group_check=True)
                    nc.scalar.activation(out=dst[:, c, j0:j1], in_=pp[:, 0:jw],
                                         func=AF.Identity,
                                         bias=bT[:, c:c + 1], scale=1.0)

        wqkpool.release()

        # V weights (loaded while QK projects), then V tiles
        ei = 0
        for (W, wt) in ((Wv, wv), (Wo, wo)):
            for k in range(NK):
                wst = wstpool.tile([128, D], f32, tag="wst",
                                   name=f"wstv_{wt.tensor.name}_{k}")
                nc.gpsimd.dma_start(out=wst, in_=W[k * 128:(k + 1) * 128, :])
                cast_copy(ca[ei % 2], wt[:, k], wst)
                ei += 1
        for b in range(BL):
            for t in range(NT):
                sz = 128 if t < NT - 1 else LAST
                sl = slice(b * SQ + t * 128, b * SQ + t * 128 + sz)
                vp = ps1.tile([128, D], f32, tag="vp", bufs=2, name=f"vp_{b}_{t}")
                for k in range(NK):
                    nc.tensor.matmul(vp[0:sz, 0:512], xT[:, k, sl], wv[:, k, 0:512],
                                     start=(k == 0), stop=False, skip_group_check=True)
                    nc.tensor.matmul(vp[0:sz, 512:768], xT[:, k, sl], wv[:, k, 512:768],
                                     start=(k == 0), stop=False, skip_group_check=True)
                nc.tensor.matmul(vp[0:sz, 0:512], onesb[0:1, 0:sz], bvb[0:1, 0:512],
                                 start=False, stop=True, skip_group_check=True)
                nc.tensor.matmul(vp[0:sz, 512:768], onesb[0:1, 0:sz], bvb[0:1, 512:768],
                                 start=False, stop=True, skip_group_check=True)
                nc.vector.tensor_copy(
                    out=vaugs[b][0:sz, t, :, 0:64],
                    in_=vp[0:sz].rearrange("p (h d) -> p h d", h=H))

    # ================= Phase 2: attention per (batch, head) =================
    stats = []
    for b in range(BL):
        st = stpool.tile([64, 2 * H], f32, tag=f"stats{b}", name=f"stats{b}")
        nc.gpsimd.memset(st, 0.0)
        stats.append(st)
    ctx_fulls = [None, None]
    scrs = [None, None]
    with tc.tile_pool(name="ps_att", bufs=1, space="PSUM") as ap_:
        for b in range(BL):
            vaug = vaugs[b]
            ctx_full = ctxpool.tile([64, H, SQ], bf16, tag="ctx_full", bufs=1,
                                    name=f"ctx_full_{b}")
            ctx_fulls[b] = ctx_full
            pend = []
            for h in range(H):
                hb = (h % 2) * 64
                hc = h // 2
                c1 = ap_.tile([65, SQ], f32, tag="c1", name=f"c1_{b}_{h}")
                c2 = ap_.tile([65, SQ], f32, tag="c2", name=f"c2_{b}_{h}")
                es = []
                for kp in range(NT):
                    kpsz = 128 if kp < NT - 1 else LAST
                    ekp = []
                    for sf in range(2):
                        kc = NK * sf + hc
                        KTs = KT[hb:hb + 64, kc, b * SQ + kp * 128:b * SQ + kp * 128 + kpsz]
                        sc = ap_.tile([128, SQ], f32, tag="sc", bufs=2,
                                      name=f"sc_{b}_{h}_{kp}_{sf}")
                        nc.tensor.matmul(sc[0:kpsz, 0:512], KTs,
                                         QT[hb:hb + 64, kc, b * SQ:b * SQ + 512],
                                         start=True, stop=True, skip_group_check=True)
                        nc.tensor.matmul(sc[0:kpsz, 512:SQ], KTs,
                                         QT[hb:hb + 64, kc, b * SQ + 512:b * SQ + SQ],
                                         start=True, stop=True, skip_group_check=True)
                        e = epool.tile([128, SQ], bf16, tag=f"e{sf}", bufs=3,
                                       name=f"e_{b}_{h}_{kp}_{sf}")
                        nc.scalar.activation(out=e[0:kpsz, :], in_=sc[0:kpsz, :],
                                             func=AF.Exp, scale=0.125)
                        ekp.append(e)
                    es.append((kp, kpsz, ekp))
                    # software pipeline: emit ctx matmuls two kp behind scores
                    if len(es) >= 3:
                        emit_ctx(nc, c1, c2, vaug, h, es.pop(0))
                while es:
                    emit_ctx(nc, c1, c2, vaug, h, es.pop(0))

                # drain psum early (frees c1/c2 banks for the next head) and
                # kick off the denominator broadcasts; the combine itself is
                # emitted one head behind so DVE never stalls on the DMA.
                c1s = cpool.tile([65, SQ], f32, tag="c1s", name=f"c1s_{b}_{h}")
                c2s = cpool.tile([65, SQ], f32, tag="c2s", name=f"c2s_{b}_{h}")
                nc.vector.tensor_copy(out=c1s, in_=c1)
                nc.vector.tensor_copy(out=c2s, in_=c2)
                d1b = rpool.tile([64, SQ], f32, tag="d1b", bufs=2, name=f"d1b_{b}_{h}")
                d2b = rpool.tile([64, SQ], f32, tag="d2b", bufs=2, name=f"d2b_{b}_{h}")
                nc.gpsimd.dma_start(out=d1b[:, 0:577],
                                    in_=bcast_ap(c1s[64:65, 0:577], 64))
                nc.gpsimd.dma_start(out=d2b[:, 0:577],
                                    in_=bcast_ap(c2s[64:65, 0:577], 64))
                pend.append((h, c1s, c2s, d1b, d2b))
                if len(pend) >= 2:
                    emit_combine(nc, rpool, cpool, ctx_full, stats[b], lam, b,
                                 pend.pop(0))
            while pend:
                emit_combine(nc, rpool, cpool, ctx_full, stats[b], lam, b,
                             pend.pop(0))

            # ---- GroupNorm for batch b ----
            sps = ap_.tile([1, 2 * H], f32, tag="sc", bufs=2, name=f"sps_{b}")
            nc.tensor.matmul(sps, ones64, stats[b], start=True, stop=True,
                             skip_group_check=True)
            ssb = stpool.tile([1, 2 * H], f32, tag=f"ssb{b}", name=f"ssb_{b}")
            nc.vector.tensor_copy(ssb, sps)
            mu = stpool.tile([1, H], f32, tag=f"mu{b}", name=f"mu_{b}")
            nc.vector.tensor_scalar(out=mu, in0=ssb[0:1, 0:H], scalar1=1.0 / GN_N,
                                    scalar2=None, op0=OP.mult)
            musq = stpool.tile([1, H], f32, tag=f"musq{b}", name=f"musq_{b}")
            nc.vector.tensor_tensor(out=musq, in0=mu, in1=mu, op=OP.mult)
            var = stpool.tile([1, H], f32, tag=f"var{b}", name=f"var_{b}")
            nc.vector.scalar_tensor_tensor(out=var, in0=ssb[0:1, H:2 * H],
                                           scalar=1.0 / GN_N, in1=musq,
                                           op0=OP.mult, op1=OP.subtract)
            lnv = stpool.tile([1, H], f32, tag=f"lnv{b}", name=f"lnv_{b}")
            nc.scalar.activation(out=lnv, in_=var, func=AF.Ln, bias=eps_t, scale=1.0)
            rstd = stpool.tile([1, H], f32, tag=f"rstd{b}", name=f"rstd_{b}")
            nc.scalar.activation(out=rstd, in_=lnv, func=AF.Exp, scale=-0.5)
            mu_b = stpool.tile([64, H], f32, tag=f"mu_b{b}", name=f"mu_b_{b}")
            rstd_b = stpool.tile([64, H], f32, tag=f"rstd_b{b}", name=f"rstd_b_{b}")
            nc.gpsimd.dma_start(out=mu_b, in_=bcast_ap(mu[0:1, :], 64))
            nc.gpsimd.dma_start(out=rstd_b, in_=bcast_ap(rstd[0:1, :], 64))
            scale_all = stpool.tile([64, H], f32, tag=f"scale_all{b}",
                                    name=f"scale_all_{b}")
            nc.vector.tensor_tensor(out=scale_all, in0=rstd_b, in1=gn_wT, op=OP.mult)
            bias_all = stpool.tile([64, H], f32, tag=f"bias_all{b}",
                                   name=f"bias_all_{b}")
            nc.vector.scalar_tensor_tensor(out=bias_all, in0=mu_b, scalar=-1.0,
                                           in1=scale_all, op0=OP.mult, op1=OP.mult)
            nc.vector.tensor_tensor(out=bias_all, in0=bias_all, in1=gn_bT, op=OP.add)
            for h in range(H):
                nc.vector.tensor_scalar(
                    out=ctx_full[:, h, 0:577], in0=ctx_full[:, h, 0:577],
                    scalar1=scale_all[:, h:h + 1], scalar2=bias_all[:, h:h + 1],
                    op0=OP.mult, op1=OP.add)
            scr = drpool.tile([D, S], bf16, tag=f"scr{b}", name=f"scr_{b}")
            scrs[b] = scr
            nc.sync.dma_start(
                out=bass.AP(tensor=scr.tensor, offset=scr.offset,
                            ap=[[S, 64], [64 * S, H], [1, S]]),
                in_=ctx_full[:, :, 0:577])

    # ================= Phase 3: reinterpret + output projection =================
    ctxTT = big.tile([128, NK, SQ2], bf16, tag="xT_ctxTT", name="ctxTT")
    with tc.tile_pool(name="ps_tp", bufs=1, space="PSUM") as ps_tp, \
         tc.tile_pool(name="cnpool", bufs=2) as cnpool:
        for b in range(BL):
            scr = scrs[b]
            for i in range(NT):
                sz = 128 if i < NT - 1 else LAST
                cn = cnpool.tile([128, D], bf16, tag="cn", name=f"cn_{b}_{i}")
                nc.sync.dma_start(
                    out=cn[0:sz, :],
                    in_=bass.AP(tensor=scr.tensor, offset=scr.offset + i * 128 * D,
                                ap=[[D, sz], [1, D]]))
                tp = ps_tp.tile([128, NK, 128], bf16, tag="tp", bufs=2,
                                name=f"tp3_{b}_{i}")
                for j in range(NK):
                    nc.tensor.transpose(tp[:, j, :], cn[:, j * 128:(j + 1) * 128],
                                        identb)
                nc.vector.tensor_copy(
                    out=ctxTT[:, :, b * SQ + i * 128:b * SQ + i * 128 + sz],
                    in_=tp[:, :, 0:sz])

    with tc.tile_pool(name="ps_o", bufs=1, space="PSUM") as ps_o:
        for b in range(BL):
            for i in range(NT):
                sz = 128 if i < NT - 1 else LAST
                sl = slice(b * SQ + i * 128, b * SQ + i * 128 + sz)
                op = ps_o.tile([128, D], f32, tag="op", bufs=2, name=f"op_{b}_{i}")
                for j in range(NK):
                    nc.tensor.matmul(op[0:sz, 0:512], ctxTT[:, j, sl], wo[:, j, 0:512],
                                     start=(j == 0), stop=False, skip_group_check=True)
                    nc.tensor.matmul(op[0:sz, 512:768], ctxTT[:, j, sl], wo[:, j, 512:768],
                                     start=(j == 0), stop=False, skip_group_check=True)
                nc.tensor.matmul(op[0:sz, 0:512], onesb[0:1, 0:sz], bob[0:1, 0:512],
                                 start=False, stop=True, skip_group_check=True)
                nc.tensor.matmul(op[0:sz, 512:768], onesb[0:1, 0:sz], bob[0:1, 512:768],
                                 start=False, stop=True, skip_group_check=True)
                ot = opool.tile([128, D], f32, tag="ot", name=f"ot_{b}_{i}")
                nc.scalar.copy(out=ot[0:sz, :], in_=op[0:sz, :])
                nc.sync.dma_start(out=out[b, i * 128:i * 128 + sz, :], in_=ot[0:sz, :])

    for p in (drpool, opool, stpool, ctxpool, rpool, cpool, epool, vpool,
              big, wpool, sing):
        p.release()


def emit_combine(nc, rpool, cpool, ctx_full, stats_b, lam, b, item):
    h, c1s, c2s, d1b, d2b = item
    r1b = rpool.tile([64, SQ], f32, tag="r1b", bufs=1, name=f"r1b_{b}_{h}")
    r2b = rpool.tile([64, SQ], f32, tag="r2b", bufs=1, name=f"r2b_{b}_{h}")
    nc.vector.reciprocal_approx_fast(out=r1b[:, 0:577], in_=d1b[:, 0:577])
    nc.vector.reciprocal_approx_fast(out=r2b[:, 0:577], in_=d2b[:, 0:577])
    ut = cpool.tile([64, SQ], bf16, tag="ut", bufs=1, name=f"ut_{b}_{h}")
    tt = cpool.tile([64, SQ], bf16, tag="tt", bufs=1, name=f"tt_{b}_{h}")
    ctxT = ctx_full[:, h, :]
    nc.vector.scalar_tensor_tensor(
        out=ut[:, 0:577], in0=c2s[0:64, 0:577], scalar=-lam,
        in1=r2b[:, 0:577], op0=OP.mult, op1=OP.mult)
    nc.vector.tensor_tensor(
        out=tt[:, 0:577], in0=c1s[0:64, 0:577], in1=r1b[:, 0:577], op=OP.mult)
    nc.vector.scalar_tensor_tensor(
        out=ctxT[:, 0:577], in0=tt[:, 0:577], scalar=1.0,
        in1=ut[:, 0:577], op0=OP.mult, op1=OP.add,
        accum_out=stats_b[:, h:h + 1])
    sqt = cpool.tile([64, SQ], bf16, tag="ut", bufs=1, name=f"sqt_{b}_{h}")
    nc.vector.scalar_tensor_tensor(
        out=sqt[:, 0:577], in0=ctxT[:, 0:577], scalar=1.0,
        in1=ctxT[:, 0:577], op0=OP.mult, op1=OP.mult,
        accum_out=stats_b[:, H + h:H + h + 1])


def emit_ctx(nc, c1, c2, vaug, h, item):
    kp, kpsz, (e1, e2) = item
    va = vaug[0:kpsz, kp, h, :]
    start = (kp == 0)
    stop = (kp == NT - 1)
    nc.tensor.matmul(c1[:, 0:512], va, e1[0:kpsz, 0:512],
                     start=start, stop=False, skip_group_check=True)
    nc.tensor.matmul(c1[:, 512:577], va, e1[0:kpsz, 512:577],
                     start=start, stop=stop, skip_group_check=True)
    nc.tensor.matmul(c2[:, 0:512], va, e2[0:kpsz, 0:512],
                     start=start, stop=False, skip_group_check=True)
    nc.tensor.matmul(c2[:, 512:577], va, e2[0:kpsz, 512:577],
                     start=start, stop=stop, skip_group_check=True)


_CACHE = {}


def _get_program(lam: float):
    key = round(float(lam), 8)
    if key not in _CACHE:
        _CACHE[key] = build_program(float(lam))
    return _CACHE[key]


LAST_EXEC_NS = 0
LAST_RESULT = None


def kernel(**inputs):
    global LAST_EXEC_NS, LAST_RESULT
    import os
    x = np.ascontiguousarray(np.asarray(inputs["x"], dtype=np.float32))
    lam = float(np.asarray(inputs["lam"]))
    nc = _get_program(lam)
    names = ["Wq", "bq", "Wk", "bk", "Wv", "bv", "Wo", "bo", "gn_w", "gn_b"]
    shared = {n: np.ascontiguousarray(np.asarray(inputs[n], dtype=np.float32))
              for n in names}
    in_maps = []
    for c in range(N_CORES):
        m = dict(shared)
        m["x"] = x[c * BL:(c + 1) * BL]
        in_maps.append(m)
    trace = bool(os.environ.get("KERNEL_TRACE"))
    res = bass_utils.run_bass_kernel_spmd(nc, in_maps, list(range(N_CORES)),
                                          trace=trace)
    if res.exec_time_ns:
        LAST_EXEC_NS = res.exec_time_ns
    LAST_RESULT = res
    return np.concatenate([res.results[c]["out"] for c in range(N_CORES)], axis=0)
